# revision 28
# baseline (speedup 1.0000x reference)
"""Trainium2 Bass kernel for nn_LuenbergerLDS (B=32, T=2048, N=512, M=512).

Math: the reference is a diagonal complex linear recurrence
    s_t = lam * s_{t-1} + x_t   (per batch, per n; x scalar per t broadcast)
followed by  y = Re(Winv @ s) @ C + x @ D + Do.

Since d == 1 the whole module is a causal LTI SIMO filter:
    y[t, b, m] = sum_{j>=0} H[j, m] * x[t - j, b] + Do[m]
with impulse response (computed on host in float64)
    H[j, m] = sum_n Re(lam_n^j) * A_re[n, m] - Im(lam_n^j) * A_im[n, m]
    A_re = Re(Winv)^T @ C,  A_im = Im(Winv)^T @ C,  H[0] += D.
A window of NLAG*128 = 512 lags truncates at 8.5e-4 of max|y| (measured
exactly on the reference data; gate is 2e-2).

Device work (per core, data-parallel over batch: 4 batches/core): pure
fp8(e4m3) matmuls in DoubleRow perf mode (2 contraction slots per
partition, 0.5 cycles/row -> 2x f32r throughput). For output chunk
t0..t0+127, the stationary operand packs TWO lag tiles' Toeplitz
diagonal slices of a zero-padded, pre-diagonalized x buffer (built on
host, so DMA loads are contiguous); the moving operand packs the two
matching row-flipped H tiles (128x(2*512)). 3 DoubleRow matmuls per
chunk: lags(0,1)_hi, lags(2,3)_hi, and a Dekker compensation pair
(x_lo*H_hi0 + x_hi*H_lo0) that fixes the head tile's fp8 quantization.

Scaling: x ops are x*32, H ops are H*16, so PSUM holds y*512 (absmax
39.3k < fp16 max). Device just downcasts PSUM to fp16 and stores; the
host divides by 512, adds Do, and upcasts to f32. Measured end-to-end
error of this exact pipeline on the reference data: 3.9e-3.
"""

import sys

sys.path.insert(0, "/opt/trn_rl_repo")

import numpy as np
import ml_dtypes

E4 = ml_dtypes.float8_e4m3

# problem dims (hardcoded per harness contract)
B, T, N, M = 32, 2048, 512, 512
NCORES = 8
BLOC = B // NCORES          # batches per core
NLAG = 4                    # lag window = NLAG*128 = 512
MODE = "fp8dr"
SX, SH = 32.0, 16.0         # operand scales; PSUM = y * SX*SH

RPAD = 128 * NLAG - 1       # 511 zero rows ahead of x in xpad
XPLEN = RPAD + T            # 2559
ND = (T + 128 * NLAG - 128) // 128   # 19 diagonal slices (d=0..18)
NZERO = NLAG - 1            # slices d<3 are all zero padding
NREAL = ND - NZERO          # 16 slices carried in DRAM (k=0..15)
TCH = T // 128              # 16 output chunks per batch
RANK = 32                   # SVD rank of the far-tail (lags 256..511)
NCB = 128 - RANK            # compB coverage: H0_lo lags 0..NCB-1


def build_program():
    """Build + compile the (SPMD, per-core) Bass program."""
    import concourse.tile as tile
    from concourse import bacc, mybir

    f32 = mybir.dt.float32
    f16 = mybir.dt.float16
    f8 = mybir.dt.float8e4
    DR = mybir.MatmulPerfMode.DoubleRow

    # free-dim element counts of the packed x tensor: [k, kind, col, b]
    # kind: 0 = x_lo, 1 = x_hi; slot k holds diagonal slice d = 18-k
    KSTR = 2 * 128 * BLOC   # 1024 elements per k slot
    SC = 4                  # output chunks batched per store (4KB DMA lines)
    nc = bacc.Bacc("TRN2", target_bir_lowering=False, debug=False)
    xall_t = nc.dram_tensor("xall", [128, NREAL * KSTR], f8, kind="ExternalInput")
    r1_t = nc.dram_tensor("r1", [128, 2 * M], f8, kind="ExternalInput")
    r2_t = nc.dram_tensor("r2", [128, 2 * M], f8, kind="ExternalInput")
    rc_t = nc.dram_tensor("rc", [128, 2 * M], f8, kind="ExternalInput")
    ut_t = nc.dram_tensor("ut", [128, 2 * RANK], f8, kind="ExternalInput")
    rb_t = nc.dram_tensor("rb", [128, 2 * M], f8, kind="ExternalInput")
    # y stored chunk-batched: [b, q, p, c*M]; host untransposes
    y_t = nc.dram_tensor(
        "y", [BLOC, TCH // SC, 128, SC * M], f16, kind="ExternalOutput"
    )

    # SBUF granules over k so dependency tracking is fine-grained and a
    # matmul's (k, k+1) pair never crosses a granule: granule g covers
    # k in [GLO[g], GHI[g]] inclusive, boundary slots duplicated.
    # Slices k>15 (d<3) are all-zero: their matmuls are skipped instead.
    GLO = [12, 8, 4, 0]
    GHI = [15, 12, 8, 4]

    def gran_of(k):                                # granule for pair (k, k+1)
        for g in range(4):
            if k >= GLO[g]:
                return g, k - GLO[g]
        raise AssertionError

    with tile.TileContext(nc) as tc:
        with (
            tc.tile_pool(name="xs", bufs=1) as xpool,
            tc.tile_pool(name="w", bufs=1) as wpool,
            tc.tile_pool(name="psum", bufs=5, space="PSUM") as psum_pool,
            tc.tile_pool(name="gsum", bufs=3, space="PSUM") as gsum_pool,
            tc.tile_pool(name="cmb", bufs=4) as cmb_pool,
            tc.tile_pool(name="out", bufs=8) as out_pool,
        ):
            load_eng = [nc.sync, nc.scalar, nc.gpsimd]

            xg = [
                xpool.tile([128, (GHI[g] - GLO[g] + 1) * KSTR], f8,
                           tag=f"xg{g}", name=f"xg{g}")
                for g in range(4)
            ]
            # tci=0 needs k15 + r1 + rc: one per queue, first in line
            nc.sync.dma_start(
                xg[0][:, 3 * KSTR : 4 * KSTR], xall_t.ap()[:, 15 * KSTR :]
            )
            r1 = wpool.tile([128, 2 * M], f8, tag="r1")
            nc.scalar.dma_start(r1[:], r1_t.ap())
            rc = wpool.tile([128, 2 * M], f8, tag="rc")
            nc.gpsimd.dma_start(rc[:], rc_t.ap())
            r2 = wpool.tile([128, 2 * M], f8, tag="r2")
            nc.scalar.dma_start(r2[:], r2_t.ap())
            ut = wpool.tile([128, 2 * RANK], f8, tag="ut")
            nc.gpsimd.dma_start(ut[:], ut_t.ap())
            rb = wpool.tile([128, 2 * M], f8, tag="rb")
            nc.sync.dma_start(rb[:], rb_t.ap())

            # remaining slot loads in consumption order, software-pipelined
            # into the b=0 group loop below
            loads = []
            for g in range(4):
                hi = GHI[g] - GLO[g] if g > 0 else GHI[g] - GLO[g] - 1
                for lo in range(hi, -1, -1):
                    loads.append((g, lo))
            li = 0

            def issue_load(n):
                nonlocal li
                for _ in range(n):
                    if li >= len(loads):
                        return
                    g, lo = loads[li]
                    a = lo * KSTR
                    da = (GLO[g] + lo) * KSTR
                    load_eng[li % 3].dma_start(
                        xg[g][:, a : a + KSTR], xall_t.ap()[:, da : da + KSTR]
                    )
                    li += 1

            issue_load(3)                           # g0: k14..k12

            xgr = [
                xg[g][:].rearrange(
                    "p (k kind col b) -> p k kind col b",
                    k=GHI[g] - GLO[g] + 1, kind=2, col=128, b=BLOC,
                )
                for g in range(4)
            ]
            r1v = r1[:].rearrange("p (s m) -> p s m", s=2)
            r2v = r2[:].rearrange("p (s m) -> p s m", s=2)
            rcv = rc[:].rearrange("p (s m) -> p s m", s=2)
            utv = ut[:].rearrange("p (s r) -> p s r", s=2)
            rbv = rb[:].rearrange("p (s m) -> p s m", s=2)

            gi = 0
            ot = [None] * BLOC
            pending = None                          # (ps, cmbv, b, tci, gi)

            def evac(ps, pb, ptci, pgi):
                c = ptci % SC
                if c == 0:
                    ot[pb] = out_pool.tile([128, SC * M], f16, name=f"ot{pb}")
                # PSUM->SBUF fp16 downcast, alternate DVE/ACT
                if (pgi % 2) == 0:
                    nc.vector.tensor_copy(ot[pb][:, c * M : (c + 1) * M], ps[:])
                else:
                    nc.scalar.copy(ot[pb][:, c * M : (c + 1) * M], ps[:])
                if c == SC - 1:
                    q = ptci // SC
                    eng = nc.sync if (pgi // SC) % 2 == 0 else nc.scalar
                    eng.dma_start(y_t.ap()[pb, q, :, :], ot[pb][:])

            def finish_pending():
                nonlocal pending
                if pending is None:
                    return
                ps, cmbv, pb, ptci, pgi = pending
                pending = None
                # MM_B: slot0 = x_lo (full compA), slot1 = g|x_hi-half
                # against rb = [Hf0_hi | V,Hf0_lo-half]
                nc.tensor.matmul(
                    ps[:], lhsT=cmbv, rhs=rbv,
                    start=False, stop=True, perf_mode=DR,
                )
                evac(ps, pb, ptci, pgi)

            for b in range(BLOC):
                for tci in range(TCH):
                    if b == 0 and tci < 8:          # prefetch 2 slots/group
                        issue_load(2)
                    k1 = (NREAL - 1) - tci          # lag tiles (0,1)
                    k2 = k1 + 2                     # lag tiles (2,3)
                    g1, l1 = gran_of(k1)
                    ps = psum_pool.tile([128, M], f32)
                    if tci == 0:                    # lag1 slice is zero pad
                        nc.tensor.matmul(
                            ps[:], lhsT=xgr[g1][:, l1, 1, :, b],
                            rhs=r1v[:, 0, :], start=True, stop=False,
                        )
                    else:
                        nc.tensor.matmul(
                            ps[:], lhsT=xgr[g1][:, l1 : l1 + 2, 1, :, b],
                            rhs=r1v, start=True, stop=False, perf_mode=DR,
                        )
                    if tci < 3:
                        # early chunks: far tail mostly zero padding; use
                        # the plain comp pair (and single lag-2 MM at tci=2)
                        if tci == 2:
                            g2, l2 = gran_of(NREAL - 1)
                            nc.tensor.matmul(
                                ps[:], lhsT=xgr[g2][:, l2, 1, :, b],
                                rhs=r2v[:, 0, :], start=False, stop=False,
                            )
                        finish_pending()
                        nc.tensor.matmul(
                            ps[:], lhsT=xgr[g1][:, l1, :, :, b],
                            rhs=rcv, start=False, stop=True, perf_mode=DR,
                        )
                        evac(ps, b, tci, gi)
                    else:
                        # far tail (lags 256..511) via rank-RANK projection:
                        # g = U^T-conv, then V-spread shares MM_B's slot1
                        g2, l2 = gran_of(k2)
                        gps = gsum_pool.tile([RANK, 128], f32)
                        nc.tensor.matmul(
                            gps[:], lhsT=utv,
                            rhs=xgr[g2][:, l2 : l2 + 2, 1, :, b],
                            start=True, stop=True, perf_mode=DR,
                        )
                        cmbt = cmb_pool.tile([128, 256], f8, name="cmbt")
                        nc.gpsimd.tensor_copy(
                            cmbt[:, :128], xgr[g1][:, l1, 0, :, b]
                        )
                        nc.vector.tensor_scalar_mul(
                            cmbt[:RANK, 128:], gps[:], 2.0 ** -7
                        )
                        # partition-group rule: start=32 spans <=32, so split
                        nc.gpsimd.tensor_copy(
                            cmbt[RANK:64, 128:], xgr[g1][RANK:64, l1, 1, :, b]
                        )
                        nc.gpsimd.tensor_copy(
                            cmbt[64:, 128:], xgr[g1][64:, l1, 1, :, b]
                        )
                        finish_pending()
                        pending = (
                            ps,
                            cmbt[:].rearrange("p (s c) -> p s c", s=2),
                            b, tci, gi,
                        )
                    gi += 1
            finish_pending()

    nc.compile()
    return nc


def _impulse_f64(lnl_re, lnl_im, W_r, W_i, C, D):
    lnl = lnl_re.astype(np.float64) + 1j * lnl_im.astype(np.float64)
    W = W_r.astype(np.float64) + 1j * W_i.astype(np.float64)
    Winv = np.linalg.inv(W)
    A_re = np.ascontiguousarray(Winv.real.T) @ C.astype(np.float64)
    A_im = np.ascontiguousarray(Winv.imag.T) @ C.astype(np.float64)
    j = np.arange(NLAG * 128, dtype=np.float64)
    P = np.exp(np.outer(j, lnl))
    H = P.real @ A_re - P.imag @ A_im
    H[0] += D[0].astype(np.float64)
    return H                                        # (NLAG*128, M) float64


def host_weights(lnl_re, lnl_im, W_r, W_i, C, D, Do):
    """fp8 moving operands: hi lag pairs, Dekker pairs, rank-RANK tail."""
    H = _impulse_f64(lnl_re, lnl_im, W_r, W_i, C, D)
    H_hi8 = (H * SH).astype(np.float32).astype(E4)
    H_lo8 = ((H[:128] * SH).astype(np.float32) - H_hi8[:128].astype(np.float32)).astype(E4)

    def flip(tile8):                                # lag-flip within a tile
        return np.ascontiguousarray(tile8[::-1, :])

    hf = [flip(H_hi8[128 * lg : 128 * (lg + 1)]) for lg in range(NLAG)]
    hfl0 = flip(H_lo8)
    r1 = np.ascontiguousarray(np.concatenate([hf[0], hf[1]], axis=1))
    r2 = np.ascontiguousarray(np.concatenate([hf[2], hf[3]], axis=1))
    rc = np.ascontiguousarray(np.concatenate([hf[0], hfl0], axis=1))

    # far tail (lags 256..511) rank-RANK SVD factors, fp8 operands:
    # U_op = fp8(64*U) flipped per 128-lag tile and paired for DoubleRow;
    # rb slot1 = [fp8(32*S*Vt) ; Hf0_lo rows RANK:128] pairs with the
    # device-assembled [g ; x_hi] mixed stationary.
    U, S, Vt = np.linalg.svd(H[256:512], full_matrices=False)
    U_op = (U[:, :RANK] * 64.0).astype(np.float32).astype(E4)   # (256, RANK)
    V_op = (32.0 * S[:RANK, None] * Vt[:RANK]).astype(np.float32).astype(E4)
    uf = [flip(U_op[128 * i : 128 * (i + 1)]) for i in range(2)]
    ut = np.ascontiguousarray(np.concatenate(uf, axis=1))       # (128, 2*RANK)
    rb_s1 = np.concatenate([V_op, hfl0[RANK:].astype(E4)], axis=0)
    rb = np.ascontiguousarray(np.concatenate([hf[0], rb_s1], axis=1))
    return {"r1": r1, "r2": r2, "rc": rc, "ut": ut, "rb": rb}


def make_in_maps(x, weights):
    """Per-core input dict: packed diagonalized hi/lo x + H tiles."""
    x64 = x[:, :, 0].astype(np.float32)             # (B, T)
    xh8 = (x64 * SX).astype(E4)
    xl8 = (x64 * SX - xh8.astype(np.float32)).astype(E4)

    # gather index A[k, p, col] = 128*(18-k) + p + col  into xpad rows
    k = np.arange(NREAL)
    A = (128 * (ND - 1 - k))[:, None, None] + np.arange(128)[None, :, None] \
        + np.arange(128)[None, None, :]             # (16, 128, 128)

    in_maps = []
    for c in range(NCORES):
        sl = slice(c * BLOC, (c + 1) * BLOC)
        xpad = np.zeros((2, XPLEN, BLOC), E4)       # [kind][row][b]
        xpad[0, RPAD:, :] = xl8[sl].T
        xpad[1, RPAD:, :] = xh8[sl].T
        g = xpad[:, A, :]                           # (2, 16, 128, 128, BLOC)
        xa = np.ascontiguousarray(np.transpose(g, (2, 1, 0, 3, 4)))
        im = dict(weights)
        im["xall"] = xa.reshape(128, NREAL * 2 * 128 * BLOC)
        in_maps.append(im)
    return in_maps


_prog_cache = {}


def kernel(x, lnl_re, lnl_im, W_r, W_i, C, D, Do):
    from concourse.bass_utils import run_bass_kernel_spmd

    x = np.asarray(x)
    lnl_re, lnl_im = np.asarray(lnl_re), np.asarray(lnl_im)
    W_r, W_i = np.asarray(W_r), np.asarray(W_i)
    C, D, Do = np.asarray(C), np.asarray(D), np.asarray(Do)

    key = (NLAG, MODE)
    if key not in _prog_cache:
        _prog_cache[key] = build_program()
    nc = _prog_cache[key]

    weights = host_weights(lnl_re, lnl_im, W_r, W_i, C, D, Do)
    in_maps = make_in_maps(np.asarray(x, np.float32), weights)
    res = run_bass_kernel_spmd(nc, in_maps, core_ids=list(range(NCORES)))
    # device layout [bloc, q, p, c*M]: t = 512*q + 128*c + p
    y = np.concatenate([res.results[i]["y"] for i in range(NCORES)], axis=0)
    y = y.reshape(B, TCH // 4, 128, 4, M).transpose(0, 1, 3, 2, 4).reshape(B, T, M)
    y = y.astype(np.float32) * np.float32(1.0 / (SX * SH)) + Do.astype(np.float32)
    return np.ascontiguousarray(y.astype(np.float32))


# revision 29
# speedup vs baseline: 2.0375x; 2.0375x over previous
"""Trainium2 Bass kernel for nn_LuenbergerLDS (B=32, T=2048, N=512, M=512).

Math: the reference is a diagonal complex linear recurrence
    s_t = lam * s_{t-1} + x_t   (per batch, per n; x scalar per t broadcast)
followed by  y = Re(Winv @ s) @ C + x @ D + Do.

Since d == 1 the whole module is a causal LTI SIMO filter:
    y[t, b, m] = sum_{j>=0} H[j, m] * x[t - j, b] + Do[m]
with impulse response (computed on host in float64)
    H[j, m] = sum_n Re(lam_n^j) * A_re[n, m] - Im(lam_n^j) * A_im[n, m]
    A_re = Re(Winv)^T @ C,  A_im = Im(Winv)^T @ C,  H[0] += D.
A window of NLAG*128 = 512 lags truncates at 8.5e-4 of max|y| (measured
exactly on the reference data; gate is 2e-2).

Device work (per core, data-parallel over batch: 4 batches/core): pure
fp8(e4m3) matmuls in DoubleRow perf mode (2 contraction slots per
partition, 0.5 cycles/row -> 2x f32r throughput). For output chunk
t0..t0+127, the stationary operand packs TWO lag tiles' Toeplitz
diagonal slices of a zero-padded, pre-diagonalized x buffer (built on
host, so DMA loads are contiguous); the moving operand packs the two
matching row-flipped H tiles (128x(2*512)). 3 DoubleRow matmuls per
chunk: lags(0,1)_hi, lags(2,3)_hi, and a Dekker compensation pair
(x_lo*H_hi0 + x_hi*H_lo0) that fixes the head tile's fp8 quantization.

Scaling: x ops are x*32, H ops are H*16, so PSUM holds y*512 (absmax
39.3k < fp16 max). Device just downcasts PSUM to fp16 and stores; the
host divides by 512, adds Do, and upcasts to f32. Measured end-to-end
error of this exact pipeline on the reference data: 3.9e-3.
"""

import sys

sys.path.insert(0, "/opt/trn_rl_repo")

import numpy as np
import ml_dtypes

E4 = ml_dtypes.float8_e4m3

# problem dims (hardcoded per harness contract)
B, T, N, M = 32, 2048, 512, 512
NCORES = 8
BLOC = B // NCORES          # batches per core
NLAG = 4                    # lag window = NLAG*128 = 512
MODE = "fp8dr"
SX, SH = 32.0, 16.0         # operand scales; PSUM = y * SX*SH

RPAD = 128 * NLAG - 1       # 511 zero rows ahead of x in xpad
XPLEN = RPAD + T            # 2559
ND = (T + 128 * NLAG - 128) // 128   # 19 diagonal slices (d=0..18)
NZERO = NLAG - 1            # slices d<3 are all zero padding
NREAL = ND - NZERO          # 16 slices carried in DRAM (k=0..15)
TCH = T // 128              # 16 output chunks per batch
RANK = 32                   # SVD rank of the far-tail (lags 256..511)
NCB = 128 - RANK            # compB coverage: H0_lo lags 0..NCB-1


def build_program():
    """Build + compile the (SPMD, per-core) Bass program."""
    import concourse.tile as tile
    from concourse import bacc, mybir

    f32 = mybir.dt.float32
    f16 = mybir.dt.float16
    f8 = mybir.dt.float8e4
    DR = mybir.MatmulPerfMode.DoubleRow

    # free-dim element counts of the packed x tensor: [k, kind, col, b]
    # kind: 0 = x_lo, 1 = x_hi, 2 = x_lo again (partitions 0:RANK of kind2
    # are overwritten in-place by the per-group g cast -> the MM_B pair
    # (x_hi, g|x_lo) is a single AP with no SBUF copies).
    KSTR = 3 * 128 * BLOC   # 1536 elements per k slot
    SC = 4                  # output chunks batched per store (4KB DMA lines)
    nc = bacc.Bacc("TRN2", target_bir_lowering=False, debug=False)
    xall_t = nc.dram_tensor("xall", [128, NREAL * KSTR], f8, kind="ExternalInput")
    r1_t = nc.dram_tensor("r1", [128, 2 * M], f8, kind="ExternalInput")
    r2_t = nc.dram_tensor("r2", [128, 2 * M], f8, kind="ExternalInput")
    rc_t = nc.dram_tensor("rc", [128, 2 * M], f8, kind="ExternalInput")
    ut_t = nc.dram_tensor("ut", [128, 2 * RANK], f8, kind="ExternalInput")
    rb_t = nc.dram_tensor("rb", [128, 2 * M], f8, kind="ExternalInput")
    # y stored chunk-batched: [b, q, p, c*M]; host untransposes
    y_t = nc.dram_tensor(
        "y", [BLOC, TCH // SC, 128, SC * M], f16, kind="ExternalOutput"
    )

    # SBUF granules over k so dependency tracking is fine-grained and a
    # matmul's (k, k+1) pair never crosses a granule: granule g covers
    # k in [GLO[g], GHI[g]] inclusive, boundary slots duplicated.
    # Slices k>15 (d<3) are all-zero: their matmuls are skipped instead.
    GLO = [12, 8, 4, 0]
    GHI = [15, 12, 8, 4]

    def gran_of(k):                                # granule for pair (k, k+1)
        for g in range(4):
            if k >= GLO[g]:
                return g, k - GLO[g]
        raise AssertionError

    with tile.TileContext(nc) as tc:
        with (
            tc.tile_pool(name="xs", bufs=1) as xpool,
            tc.tile_pool(name="w", bufs=1) as wpool,
            tc.tile_pool(name="psum", bufs=5, space="PSUM") as psum_pool,
            tc.tile_pool(name="gsum", bufs=3, space="PSUM") as gsum_pool,
            tc.tile_pool(name="out", bufs=8) as out_pool,
        ):
            load_eng = [nc.sync, nc.scalar, nc.gpsimd]

            xg = [
                xpool.tile([128, (GHI[g] - GLO[g] + 1) * KSTR], f8,
                           tag=f"xg{g}", name=f"xg{g}")
                for g in range(4)
            ]
            # tci=0 needs k15 + r1 + rc: one per queue, first in line
            nc.sync.dma_start(
                xg[0][:, 3 * KSTR : 4 * KSTR], xall_t.ap()[:, 15 * KSTR :]
            )
            r1 = wpool.tile([128, 2 * M], f8, tag="r1")
            nc.scalar.dma_start(r1[:], r1_t.ap())
            rc = wpool.tile([128, 2 * M], f8, tag="rc")
            nc.gpsimd.dma_start(rc[:], rc_t.ap())
            r2 = wpool.tile([128, 2 * M], f8, tag="r2")
            nc.scalar.dma_start(r2[:], r2_t.ap())
            ut = wpool.tile([128, 2 * RANK], f8, tag="ut")
            nc.gpsimd.dma_start(ut[:], ut_t.ap())
            rb = wpool.tile([128, 2 * M], f8, tag="rb")
            nc.sync.dma_start(rb[:], rb_t.ap())

            # remaining slot loads in consumption order, software-pipelined
            # into the b=0 group loop below
            loads = []
            for g in range(4):
                hi = GHI[g] - GLO[g] if g > 0 else GHI[g] - GLO[g] - 1
                for lo in range(hi, -1, -1):
                    loads.append((g, lo))
            li = 0

            def issue_load(n):
                nonlocal li
                for _ in range(n):
                    if li >= len(loads):
                        return
                    g, lo = loads[li]
                    a = lo * KSTR
                    da = (GLO[g] + lo) * KSTR
                    load_eng[li % 3].dma_start(
                        xg[g][:, a : a + KSTR], xall_t.ap()[:, da : da + KSTR]
                    )
                    li += 1

            issue_load(3)                           # g0: k14..k12

            xgr = [
                xg[g][:].rearrange(
                    "p (k kind col b) -> p k kind col b",
                    k=GHI[g] - GLO[g] + 1, kind=3, col=128, b=BLOC,
                )
                for g in range(4)
            ]
            r1v = r1[:].rearrange("p (s m) -> p s m", s=2)
            r2v = r2[:].rearrange("p (s m) -> p s m", s=2)
            rcv = rc[:].rearrange("p (s m) -> p s m", s=2)
            utv = ut[:].rearrange("p (s r) -> p s r", s=2)
            rbv = rb[:].rearrange("p (s m) -> p s m", s=2)

            gi = 0
            ot = [None] * BLOC
            pending = None                          # (ps, cmbv, b, tci, gi)

            def evac(ps, pb, ptci, pgi):
                c = ptci % SC
                if c == 0:
                    ot[pb] = out_pool.tile([128, SC * M], f16, name=f"ot{pb}")
                # PSUM->SBUF fp16 downcast, alternate DVE/ACT
                if (pgi % 2) == 0:
                    nc.vector.tensor_copy(ot[pb][:, c * M : (c + 1) * M], ps[:])
                else:
                    nc.scalar.copy(ot[pb][:, c * M : (c + 1) * M], ps[:])
                if c == SC - 1:
                    q = ptci // SC
                    eng = nc.sync if (pgi // SC) % 2 == 0 else nc.scalar
                    eng.dma_start(y_t.ap()[pb, q, :, :], ot[pb][:])

            def finish_pending():
                nonlocal pending
                if pending is None:
                    return
                ps, cmbv, pb, ptci, pgi = pending
                pending = None
                # MM_B: slot0 = x_lo (full compA), slot1 = g|x_hi-half
                # against rb = [Hf0_hi | V,Hf0_lo-half]
                nc.tensor.matmul(
                    ps[:], lhsT=cmbv, rhs=rbv,
                    start=False, stop=True, perf_mode=DR,
                )
                evac(ps, pb, ptci, pgi)

            for b in range(BLOC):
                for tci in range(TCH):
                    if b == 0 and tci < 8:          # prefetch 2 slots/group
                        issue_load(2)
                    k1 = (NREAL - 1) - tci          # lag tiles (0,1)
                    k2 = k1 + 2                     # lag tiles (2,3)
                    g1, l1 = gran_of(k1)
                    ps = psum_pool.tile([128, M], f32)
                    if tci == 0:                    # lag1 slice is zero pad
                        nc.tensor.matmul(
                            ps[:], lhsT=xgr[g1][:, l1, 1, :, b],
                            rhs=r1v[:, 0, :], start=True, stop=False,
                        )
                    else:
                        nc.tensor.matmul(
                            ps[:], lhsT=xgr[g1][:, l1 : l1 + 2, 1, :, b],
                            rhs=r1v, start=True, stop=False, perf_mode=DR,
                        )
                    if tci < 3:
                        # early chunks: far tail mostly zero padding; use
                        # the plain comp pair (and single lag-2 MM at tci=2)
                        if tci == 2:
                            g2, l2 = gran_of(NREAL - 1)
                            nc.tensor.matmul(
                                ps[:], lhsT=xgr[g2][:, l2, 1, :, b],
                                rhs=r2v[:, 0, :], start=False, stop=False,
                            )
                        finish_pending()
                        nc.tensor.matmul(
                            ps[:], lhsT=xgr[g1][:, l1, 0:2, :, b],
                            rhs=rcv, start=False, stop=True, perf_mode=DR,
                        )
                        evac(ps, b, tci, gi)
                    else:
                        # far tail (lags 256..511) via rank-RANK projection:
                        # g = U^T-conv, then V-spread shares MM_B's slot1
                        g2, l2 = gran_of(k2)
                        gps = gsum_pool.tile([RANK, 128], f32)
                        nc.tensor.matmul(
                            gps[:], lhsT=utv,
                            rhs=xgr[g2][:, l2 : l2 + 2, 1, :, b],
                            start=True, stop=True, perf_mode=DR,
                        )
                        nc.vector.tensor_scalar_mul(
                            xgr[g1][:RANK, l1, 2, :, b], gps[:], 2.0 ** -7
                        )
                        finish_pending()
                        pending = (ps, xgr[g1][:, l1, 1:3, :, b], b, tci, gi)
                    gi += 1
            finish_pending()

    nc.compile()
    return nc


def _impulse_f64(lnl_re, lnl_im, W_r, W_i, C, D):
    lnl = lnl_re.astype(np.float64) + 1j * lnl_im.astype(np.float64)
    W = W_r.astype(np.float64) + 1j * W_i.astype(np.float64)
    Winv = np.linalg.inv(W)
    A_re = np.ascontiguousarray(Winv.real.T) @ C.astype(np.float64)
    A_im = np.ascontiguousarray(Winv.imag.T) @ C.astype(np.float64)
    j = np.arange(NLAG * 128, dtype=np.float64)
    P = np.exp(np.outer(j, lnl))
    H = P.real @ A_re - P.imag @ A_im
    H[0] += D[0].astype(np.float64)
    return H                                        # (NLAG*128, M) float64


def host_weights(lnl_re, lnl_im, W_r, W_i, C, D, Do):
    """fp8 moving operands: hi lag pairs, Dekker pairs, rank-RANK tail."""
    H = _impulse_f64(lnl_re, lnl_im, W_r, W_i, C, D)
    H_hi8 = (H * SH).astype(np.float32).astype(E4)
    H_lo8 = ((H[:128] * SH).astype(np.float32) - H_hi8[:128].astype(np.float32)).astype(E4)

    def flip(tile8):                                # lag-flip within a tile
        return np.ascontiguousarray(tile8[::-1, :])

    hf = [flip(H_hi8[128 * lg : 128 * (lg + 1)]) for lg in range(NLAG)]
    hfl0 = flip(H_lo8)
    r1 = np.ascontiguousarray(np.concatenate([hf[0], hf[1]], axis=1))
    r2 = np.ascontiguousarray(np.concatenate([hf[2], hf[3]], axis=1))
    rc = np.ascontiguousarray(np.concatenate([hf[0], hfl0], axis=1))

    # far tail (lags 256..511) rank-RANK SVD factors, fp8 operands:
    # U_op = fp8(64*U) flipped per 128-lag tile and paired for DoubleRow;
    # rb slot1 = [fp8(32*S*Vt) ; Hf0_lo rows RANK:128] pairs with the
    # device-assembled [g ; x_hi] mixed stationary.
    U, S, Vt = np.linalg.svd(H[256:512], full_matrices=False)
    U_op = (U[:, :RANK] * 64.0).astype(np.float32).astype(E4)   # (256, RANK)
    V_op = (32.0 * S[:RANK, None] * Vt[:RANK]).astype(np.float32).astype(E4)
    uf = [flip(U_op[128 * i : 128 * (i + 1)]) for i in range(2)]
    ut = np.ascontiguousarray(np.concatenate(uf, axis=1))       # (128, 2*RANK)
    rb_s1 = np.concatenate([V_op, hf[0][RANK:]], axis=0)
    rb = np.ascontiguousarray(np.concatenate([hfl0, rb_s1], axis=1))
    return {"r1": r1, "r2": r2, "rc": rc, "ut": ut, "rb": rb}


def make_in_maps(x, weights):
    """Per-core input dict: packed diagonalized hi/lo x + H tiles."""
    x64 = x[:, :, 0].astype(np.float32)             # (B, T)
    xh8 = (x64 * SX).astype(E4)
    xl8 = (x64 * SX - xh8.astype(np.float32)).astype(E4)

    # gather index A[k, p, col] = 128*(18-k) + p + col  into xpad rows
    k = np.arange(NREAL)
    A = (128 * (ND - 1 - k))[:, None, None] + np.arange(128)[None, :, None] \
        + np.arange(128)[None, None, :]             # (16, 128, 128)

    in_maps = []
    for c in range(NCORES):
        sl = slice(c * BLOC, (c + 1) * BLOC)
        xpad = np.zeros((3, XPLEN, BLOC), E4)       # [kind][row][b]
        xpad[0, RPAD:, :] = xl8[sl].T
        xpad[1, RPAD:, :] = xh8[sl].T
        xpad[2, RPAD:, :] = xl8[sl].T
        g = xpad[:, A, :]                           # (3, 16, 128, 128, BLOC)
        xa = np.ascontiguousarray(np.transpose(g, (2, 1, 0, 3, 4)))
        im = dict(weights)
        im["xall"] = xa.reshape(128, NREAL * 3 * 128 * BLOC)
        in_maps.append(im)
    return in_maps


_prog_cache = {}


def kernel(x, lnl_re, lnl_im, W_r, W_i, C, D, Do):
    from concourse.bass_utils import run_bass_kernel_spmd

    x = np.asarray(x)
    lnl_re, lnl_im = np.asarray(lnl_re), np.asarray(lnl_im)
    W_r, W_i = np.asarray(W_r), np.asarray(W_i)
    C, D, Do = np.asarray(C), np.asarray(D), np.asarray(Do)

    key = (NLAG, MODE)
    if key not in _prog_cache:
        _prog_cache[key] = build_program()
    nc = _prog_cache[key]

    weights = host_weights(lnl_re, lnl_im, W_r, W_i, C, D, Do)
    in_maps = make_in_maps(np.asarray(x, np.float32), weights)
    res = run_bass_kernel_spmd(nc, in_maps, core_ids=list(range(NCORES)))
    # device layout [bloc, q, p, c*M]: t = 512*q + 128*c + p
    y = np.concatenate([res.results[i]["y"] for i in range(NCORES)], axis=0)
    y = y.reshape(B, TCH // 4, 128, 4, M).transpose(0, 1, 3, 2, 4).reshape(B, T, M)
    y = y.astype(np.float32) * np.float32(1.0 / (SX * SH)) + Do.astype(np.float32)
    return np.ascontiguousarray(y.astype(np.float32))


# revision 30
# speedup vs baseline: 2.0604x; 1.0113x over previous
"""Trainium2 Bass kernel for nn_LuenbergerLDS (B=32, T=2048, N=512, M=512).

Math: the reference is a diagonal complex linear recurrence
    s_t = lam * s_{t-1} + x_t   (per batch, per n; x scalar per t broadcast)
followed by  y = Re(Winv @ s) @ C + x @ D + Do.

Since d == 1 the whole module is a causal LTI SIMO filter:
    y[t, b, m] = sum_{j>=0} H[j, m] * x[t - j, b] + Do[m]
with impulse response (computed on host in float64)
    H[j, m] = sum_n Re(lam_n^j) * A_re[n, m] - Im(lam_n^j) * A_im[n, m]
    A_re = Re(Winv)^T @ C,  A_im = Im(Winv)^T @ C,  H[0] += D.
A window of NLAG*128 = 512 lags truncates at 8.5e-4 of max|y| (measured
exactly on the reference data; gate is 2e-2).

Device work (per core, data-parallel over batch: 4 batches/core): pure
fp8(e4m3) matmuls in DoubleRow perf mode (2 contraction slots per
partition, 0.5 cycles/row -> 2x f32r throughput). For output chunk
t0..t0+127, the stationary operand packs TWO lag tiles' Toeplitz
diagonal slices of a zero-padded, pre-diagonalized x buffer (built on
host, so DMA loads are contiguous); the moving operand packs the two
matching row-flipped H tiles (128x(2*512)). 3 DoubleRow matmuls per
chunk: lags(0,1)_hi, lags(2,3)_hi, and a Dekker compensation pair
(x_lo*H_hi0 + x_hi*H_lo0) that fixes the head tile's fp8 quantization.

Scaling: x ops are x*32, H ops are H*16, so PSUM holds y*512 (absmax
39.3k < fp16 max). Device just downcasts PSUM to fp16 and stores; the
host divides by 512, adds Do, and upcasts to f32. Measured end-to-end
error of this exact pipeline on the reference data: 3.9e-3.
"""

import sys

sys.path.insert(0, "/opt/trn_rl_repo")

import numpy as np
import ml_dtypes

E4 = ml_dtypes.float8_e4m3

# problem dims (hardcoded per harness contract)
B, T, N, M = 32, 2048, 512, 512
NCORES = 8
BLOC = B // NCORES          # batches per core
NLAG = 4                    # lag window = NLAG*128 = 512
MODE = "fp8dr"
SX, SH = 32.0, 16.0         # operand scales; PSUM = y * SX*SH

RPAD = 128 * NLAG - 1       # 511 zero rows ahead of x in xpad
XPLEN = RPAD + T            # 2559
ND = (T + 128 * NLAG - 128) // 128   # 19 diagonal slices (d=0..18)
NZERO = NLAG - 1            # slices d<3 are all zero padding
NREAL = ND - NZERO          # 16 slices carried in DRAM (k=0..15)
TCH = T // 128              # 16 output chunks per batch
RANK = 32                   # SVD rank of the far-tail (lags 256..511)
NCB = 128 - RANK            # compB coverage: H0_lo lags 0..NCB-1


def build_program():
    """Build + compile the (SPMD, per-core) Bass program."""
    import concourse.tile as tile
    from concourse import bacc, mybir

    f32 = mybir.dt.float32
    f16 = mybir.dt.float16
    f8 = mybir.dt.float8e4
    DR = mybir.MatmulPerfMode.DoubleRow

    # free-dim element counts of the packed x tensor: [k, kind, col, b]
    # kind: 0 = x_lo, 1 = x_hi, 2 = x_lo again (partitions 0:RANK of kind2
    # are overwritten in-place by the per-group g cast -> the MM_B pair
    # (x_hi, g|x_lo) is a single AP with no SBUF copies).
    KSTR = 3 * 128 * BLOC   # 1536 elements per k slot
    SC = 4                  # output chunks batched per store (4KB DMA lines)
    nc = bacc.Bacc("TRN2", target_bir_lowering=False, debug=False)
    xall_t = nc.dram_tensor("xall", [128, NREAL * KSTR], f8, kind="ExternalInput")
    r1_t = nc.dram_tensor("r1", [128, 2 * M], f8, kind="ExternalInput")
    r2_t = nc.dram_tensor("r2", [128, 2 * M], f8, kind="ExternalInput")
    rc_t = nc.dram_tensor("rc", [128, 2 * M], f8, kind="ExternalInput")
    ut_t = nc.dram_tensor("ut", [128, 2 * RANK], f8, kind="ExternalInput")
    rb_t = nc.dram_tensor("rb", [128, 2 * M], f8, kind="ExternalInput")
    # y stored chunk-batched: [b, q, p, c*M]; host untransposes
    y_t = nc.dram_tensor(
        "y", [BLOC, TCH // SC, 128, SC * M], f16, kind="ExternalOutput"
    )

    # SBUF granules over k so dependency tracking is fine-grained and a
    # matmul's (k, k+1) pair never crosses a granule: granule g covers
    # k in [GLO[g], GHI[g]] inclusive, boundary slots duplicated.
    # Slices k>15 (d<3) are all-zero: their matmuls are skipped instead.
    GLO = [12, 8, 4, 0]
    GHI = [15, 12, 8, 4]

    def gran_of(k):                                # granule for pair (k, k+1)
        for g in range(4):
            if k >= GLO[g]:
                return g, k - GLO[g]
        raise AssertionError

    with tile.TileContext(nc) as tc:
        with (
            tc.tile_pool(name="xs", bufs=1) as xpool,
            tc.tile_pool(name="w", bufs=1) as wpool,
            tc.tile_pool(name="psum", bufs=5, space="PSUM") as psum_pool,
            tc.tile_pool(name="gsum", bufs=3, space="PSUM") as gsum_pool,
            tc.tile_pool(name="out", bufs=8) as out_pool,
        ):
            load_eng = [nc.sync, nc.scalar, nc.gpsimd]

            xg = [
                xpool.tile([128, (GHI[g] - GLO[g] + 1) * KSTR], f8,
                           tag=f"xg{g}", name=f"xg{g}")
                for g in range(4)
            ]
            # tci=0 needs k15 + r1 + rc: one per queue, first in line
            nc.sync.dma_start(
                xg[0][:, 3 * KSTR : 4 * KSTR], xall_t.ap()[:, 15 * KSTR :]
            )
            r1 = wpool.tile([128, 2 * M], f8, tag="r1")
            nc.scalar.dma_start(r1[:], r1_t.ap())
            rc = wpool.tile([128, 2 * M], f8, tag="rc")
            nc.gpsimd.dma_start(rc[:], rc_t.ap())
            r2 = wpool.tile([128, 2 * M], f8, tag="r2")
            nc.scalar.dma_start(r2[:], r2_t.ap())
            ut = wpool.tile([128, 2 * RANK], f8, tag="ut")
            nc.gpsimd.dma_start(ut[:], ut_t.ap())
            rb = wpool.tile([128, 2 * M], f8, tag="rb")
            nc.sync.dma_start(rb[:], rb_t.ap())

            # remaining slot loads in consumption order, software-pipelined
            # into the b=0 group loop below
            loads = []
            for g in range(4):
                hi = GHI[g] - GLO[g] if g > 0 else GHI[g] - GLO[g] - 1
                for lo in range(hi, -1, -1):
                    loads.append((g, lo))
            li = 0

            def issue_load(n):
                nonlocal li
                for _ in range(n):
                    if li >= len(loads):
                        return
                    g, lo = loads[li]
                    a = lo * KSTR
                    da = (GLO[g] + lo) * KSTR
                    load_eng[li % 3].dma_start(
                        xg[g][:, a : a + KSTR], xall_t.ap()[:, da : da + KSTR]
                    )
                    li += 1

            issue_load(len(loads))                  # issue all, in order

            xgr = [
                xg[g][:].rearrange(
                    "p (k kind col b) -> p k kind col b",
                    k=GHI[g] - GLO[g] + 1, kind=3, col=128, b=BLOC,
                )
                for g in range(4)
            ]
            r1v = r1[:].rearrange("p (s m) -> p s m", s=2)
            r2v = r2[:].rearrange("p (s m) -> p s m", s=2)
            rcv = rc[:].rearrange("p (s m) -> p s m", s=2)
            utv = ut[:].rearrange("p (s r) -> p s r", s=2)
            rbv = rb[:].rearrange("p (s m) -> p s m", s=2)

            gi = 0
            ot = [None] * BLOC
            pending = None                          # (ps, cmbv, b, tci, gi)

            def evac(ps, pb, ptci, pgi):
                c = ptci % SC
                if c == 0:
                    ot[pb] = out_pool.tile([128, SC * M], f16, name=f"ot{pb}")
                # PSUM->SBUF fp16 downcast, alternate DVE/ACT
                if (pgi % 2) == 0:
                    nc.vector.tensor_copy(ot[pb][:, c * M : (c + 1) * M], ps[:])
                else:
                    nc.scalar.copy(ot[pb][:, c * M : (c + 1) * M], ps[:])
                if c == 1:
                    q = ptci // SC
                    eng = nc.sync if (pgi // 2) % 2 == 0 else nc.scalar
                    eng.dma_start(y_t.ap()[pb, q, :, : 2 * M],
                                  ot[pb][:, : 2 * M])
                elif c == SC - 1:
                    q = ptci // SC
                    eng = nc.sync if (pgi // 2) % 2 == 0 else nc.scalar
                    eng.dma_start(y_t.ap()[pb, q, :, 2 * M :],
                                  ot[pb][:, 2 * M :])

            def finish_pending():
                nonlocal pending
                if pending is None:
                    return
                ps, cmbv, pb, ptci, pgi = pending
                pending = None
                # MM_B: slot0 = x_lo (full compA), slot1 = g|x_hi-half
                # against rb = [Hf0_hi | V,Hf0_lo-half]
                nc.tensor.matmul(
                    ps[:], lhsT=cmbv, rhs=rbv,
                    start=False, stop=True, perf_mode=DR,
                )
                evac(ps, pb, ptci, pgi)

            for b in range(BLOC):
                for tci in range(TCH):
                    k1 = (NREAL - 1) - tci          # lag tiles (0,1)
                    k2 = k1 + 2                     # lag tiles (2,3)
                    g1, l1 = gran_of(k1)
                    ps = psum_pool.tile([128, M], f32)
                    if tci == 0:                    # lag1 slice is zero pad
                        nc.tensor.matmul(
                            ps[:], lhsT=xgr[g1][:, l1, 1, :, b],
                            rhs=r1v[:, 0, :], start=True, stop=False,
                        )
                    else:
                        nc.tensor.matmul(
                            ps[:], lhsT=xgr[g1][:, l1 : l1 + 2, 1, :, b],
                            rhs=r1v, start=True, stop=False, perf_mode=DR,
                        )
                    if tci < 3:
                        # early chunks: far tail mostly zero padding; use
                        # the plain comp pair (and single lag-2 MM at tci=2)
                        if tci == 2:
                            g2, l2 = gran_of(NREAL - 1)
                            nc.tensor.matmul(
                                ps[:], lhsT=xgr[g2][:, l2, 1, :, b],
                                rhs=r2v[:, 0, :], start=False, stop=False,
                            )
                        finish_pending()
                        nc.tensor.matmul(
                            ps[:], lhsT=xgr[g1][:, l1, 0:2, :, b],
                            rhs=rcv, start=False, stop=True, perf_mode=DR,
                        )
                        evac(ps, b, tci, gi)
                    else:
                        # far tail (lags 256..511) via rank-RANK projection:
                        # g = U^T-conv, then V-spread shares MM_B's slot1
                        g2, l2 = gran_of(k2)
                        gps = gsum_pool.tile([RANK, 128], f32)
                        nc.tensor.matmul(
                            gps[:], lhsT=utv,
                            rhs=xgr[g2][:, l2 : l2 + 2, 1, :, b],
                            start=True, stop=True, perf_mode=DR,
                        )
                        nc.vector.tensor_scalar_mul(
                            xgr[g1][:RANK, l1, 2, :, b], gps[:], 2.0 ** -7
                        )
                        finish_pending()
                        pending = (ps, xgr[g1][:, l1, 1:3, :, b], b, tci, gi)
                    gi += 1
            finish_pending()

    nc.compile()
    return nc


def _impulse_f64(lnl_re, lnl_im, W_r, W_i, C, D):
    lnl = lnl_re.astype(np.float64) + 1j * lnl_im.astype(np.float64)
    W = W_r.astype(np.float64) + 1j * W_i.astype(np.float64)
    Winv = np.linalg.inv(W)
    A_re = np.ascontiguousarray(Winv.real.T) @ C.astype(np.float64)
    A_im = np.ascontiguousarray(Winv.imag.T) @ C.astype(np.float64)
    j = np.arange(NLAG * 128, dtype=np.float64)
    P = np.exp(np.outer(j, lnl))
    H = P.real @ A_re - P.imag @ A_im
    H[0] += D[0].astype(np.float64)
    return H                                        # (NLAG*128, M) float64


def host_weights(lnl_re, lnl_im, W_r, W_i, C, D, Do):
    """fp8 moving operands: hi lag pairs, Dekker pairs, rank-RANK tail."""
    H = _impulse_f64(lnl_re, lnl_im, W_r, W_i, C, D)
    H_hi8 = (H * SH).astype(np.float32).astype(E4)
    H_lo8 = ((H[:128] * SH).astype(np.float32) - H_hi8[:128].astype(np.float32)).astype(E4)

    def flip(tile8):                                # lag-flip within a tile
        return np.ascontiguousarray(tile8[::-1, :])

    hf = [flip(H_hi8[128 * lg : 128 * (lg + 1)]) for lg in range(NLAG)]
    hfl0 = flip(H_lo8)
    r1 = np.ascontiguousarray(np.concatenate([hf[0], hf[1]], axis=1))
    r2 = np.ascontiguousarray(np.concatenate([hf[2], hf[3]], axis=1))
    rc = np.ascontiguousarray(np.concatenate([hf[0], hfl0], axis=1))

    # far tail (lags 256..511) rank-RANK SVD factors, fp8 operands:
    # U_op = fp8(64*U) flipped per 128-lag tile and paired for DoubleRow;
    # rb slot1 = [fp8(32*S*Vt) ; Hf0_lo rows RANK:128] pairs with the
    # device-assembled [g ; x_hi] mixed stationary.
    U, S, Vt = np.linalg.svd(H[256:512], full_matrices=False)
    U_op = (U[:, :RANK] * 64.0).astype(np.float32).astype(E4)   # (256, RANK)
    V_op = (32.0 * S[:RANK, None] * Vt[:RANK]).astype(np.float32).astype(E4)
    uf = [flip(U_op[128 * i : 128 * (i + 1)]) for i in range(2)]
    ut = np.ascontiguousarray(np.concatenate(uf, axis=1))       # (128, 2*RANK)
    rb_s1 = np.concatenate([V_op, hf[0][RANK:]], axis=0)
    rb = np.ascontiguousarray(np.concatenate([hfl0, rb_s1], axis=1))
    return {"r1": r1, "r2": r2, "rc": rc, "ut": ut, "rb": rb}


def make_in_maps(x, weights):
    """Per-core input dict: packed diagonalized hi/lo x + H tiles."""
    x64 = x[:, :, 0].astype(np.float32)             # (B, T)
    xh8 = (x64 * SX).astype(E4)
    xl8 = (x64 * SX - xh8.astype(np.float32)).astype(E4)

    # gather index A[k, p, col] = 128*(18-k) + p + col  into xpad rows
    k = np.arange(NREAL)
    A = (128 * (ND - 1 - k))[:, None, None] + np.arange(128)[None, :, None] \
        + np.arange(128)[None, None, :]             # (16, 128, 128)

    in_maps = []
    for c in range(NCORES):
        sl = slice(c * BLOC, (c + 1) * BLOC)
        xpad = np.zeros((3, XPLEN, BLOC), E4)       # [kind][row][b]
        xpad[0, RPAD:, :] = xl8[sl].T
        xpad[1, RPAD:, :] = xh8[sl].T
        xpad[2, RPAD:, :] = xl8[sl].T
        g = xpad[:, A, :]                           # (3, 16, 128, 128, BLOC)
        xa = np.ascontiguousarray(np.transpose(g, (2, 1, 0, 3, 4)))
        im = dict(weights)
        im["xall"] = xa.reshape(128, NREAL * 3 * 128 * BLOC)
        in_maps.append(im)
    return in_maps


_prog_cache = {}


def kernel(x, lnl_re, lnl_im, W_r, W_i, C, D, Do):
    from concourse.bass_utils import run_bass_kernel_spmd

    x = np.asarray(x)
    lnl_re, lnl_im = np.asarray(lnl_re), np.asarray(lnl_im)
    W_r, W_i = np.asarray(W_r), np.asarray(W_i)
    C, D, Do = np.asarray(C), np.asarray(D), np.asarray(Do)

    key = (NLAG, MODE)
    if key not in _prog_cache:
        _prog_cache[key] = build_program()
    nc = _prog_cache[key]

    weights = host_weights(lnl_re, lnl_im, W_r, W_i, C, D, Do)
    in_maps = make_in_maps(np.asarray(x, np.float32), weights)
    res = run_bass_kernel_spmd(nc, in_maps, core_ids=list(range(NCORES)))
    # device layout [bloc, q, p, c*M]: t = 512*q + 128*c + p
    y = np.concatenate([res.results[i]["y"] for i in range(NCORES)], axis=0)
    y = y.reshape(B, TCH // 4, 128, 4, M).transpose(0, 1, 3, 2, 4).reshape(B, T, M)
    y = y.astype(np.float32) * np.float32(1.0 / (SX * SH)) + Do.astype(np.float32)
    return np.ascontiguousarray(y.astype(np.float32))


# revision 31
# speedup vs baseline: 2.1432x; 1.0402x over previous
"""Trainium2 Bass kernel for nn_LuenbergerLDS (B=32, T=2048, N=512, M=512).

Math: the reference is a diagonal complex linear recurrence
    s_t = lam * s_{t-1} + x_t   (per batch, per n; x scalar per t broadcast)
followed by  y = Re(Winv @ s) @ C + x @ D + Do.

Since d == 1 the whole module is a causal LTI SIMO filter:
    y[t, b, m] = sum_{j>=0} H[j, m] * x[t - j, b] + Do[m]
with impulse response (computed on host in float64)
    H[j, m] = sum_n Re(lam_n^j) * A_re[n, m] - Im(lam_n^j) * A_im[n, m]
    A_re = Re(Winv)^T @ C,  A_im = Im(Winv)^T @ C,  H[0] += D.
A window of NLAG*128 = 512 lags truncates at 8.5e-4 of max|y| (measured
exactly on the reference data; gate is 2e-2).

Device work (per core, data-parallel over batch: 4 batches/core): pure
fp8(e4m3) matmuls in DoubleRow perf mode (2 contraction slots per
partition, 0.5 cycles/row -> 2x f32r throughput). For output chunk
t0..t0+127, the stationary operand packs TWO lag tiles' Toeplitz
diagonal slices of a zero-padded, pre-diagonalized x buffer (built on
host, so DMA loads are contiguous); the moving operand packs the two
matching row-flipped H tiles (128x(2*512)). 3 DoubleRow matmuls per
chunk: lags(0,1)_hi, lags(2,3)_hi, and a Dekker compensation pair
(x_lo*H_hi0 + x_hi*H_lo0) that fixes the head tile's fp8 quantization.

Scaling: x ops are x*32, H ops are H*16, so PSUM holds y*512 (absmax
39.3k < fp16 max). Device just downcasts PSUM to fp16 and stores; the
host divides by 512, adds Do, and upcasts to f32. Measured end-to-end
error of this exact pipeline on the reference data: 3.9e-3.
"""

import sys

sys.path.insert(0, "/opt/trn_rl_repo")

import numpy as np
import ml_dtypes

E4 = ml_dtypes.float8_e4m3

# problem dims (hardcoded per harness contract)
B, T, N, M = 32, 2048, 512, 512
NCORES = 8
BLOC = B // NCORES          # batches per core
NLAG = 4                    # lag window = NLAG*128 = 512
MODE = "fp8dr"
SX, SH = 32.0, 16.0         # operand scales; PSUM = y * SX*SH

RPAD = 128 * NLAG - 1       # 511 zero rows ahead of x in xpad
XPLEN = RPAD + T            # 2559
ND = (T + 128 * NLAG - 128) // 128   # 19 diagonal slices (d=0..18)
NZERO = NLAG - 1            # slices d<3 are all zero padding
NREAL = ND - NZERO          # 16 slices carried in DRAM (k=0..15)
TCH = T // 128              # 16 output chunks per batch
RANK = 32                   # SVD rank of the far-tail (lags 256..511)
NCB = 128 - RANK            # compB coverage: H0_lo lags 0..NCB-1


def build_program():
    """Build + compile the (SPMD, per-core) Bass program."""
    import concourse.tile as tile
    from concourse import bacc, mybir

    f32 = mybir.dt.float32
    f16 = mybir.dt.float16
    f8 = mybir.dt.float8e4
    DR = mybir.MatmulPerfMode.DoubleRow

    # free-dim element counts of the packed x tensor: [k, kind, col, b]
    # kind: 0 = x_hi, 1 = x_lo. For slots k<=12 the partitions 0:RANK of
    # kind1 are dead (compA only covers lags 0..95) and get overwritten
    # in-place by the per-group g cast, so the MM_B stationary pair
    # (x_hi, g|x_lo) is a single AP with no SBUF copies.
    KSTR = 2 * 128 * BLOC   # 1024 elements per k slot
    SC = 4                  # output chunks batched per store (4KB DMA lines)
    nc = bacc.Bacc("TRN2", target_bir_lowering=False, debug=False)
    xall_t = nc.dram_tensor("xall", [128, NREAL * KSTR], f8, kind="ExternalInput")
    r1_t = nc.dram_tensor("r1", [128, 2 * M], f8, kind="ExternalInput")
    r2_t = nc.dram_tensor("r2", [128, 2 * M], f8, kind="ExternalInput")
    rc_t = nc.dram_tensor("rc", [128, 2 * M], f8, kind="ExternalInput")
    ut_t = nc.dram_tensor("ut", [128, 2 * RANK], f8, kind="ExternalInput")
    rb_t = nc.dram_tensor("rb", [128, 2 * M], f8, kind="ExternalInput")
    # y stored chunk-batched: [b, q, p, c*M]; host untransposes
    y_t = nc.dram_tensor(
        "y", [BLOC, TCH // SC, 128, SC * M], f16, kind="ExternalOutput"
    )

    # SBUF granules over k so dependency tracking is fine-grained and a
    # matmul's (k, k+1) pair never crosses a granule: granule g covers
    # k in [GLO[g], GHI[g]] inclusive, boundary slots duplicated.
    # Slices k>15 (d<3) are all-zero: their matmuls are skipped instead.
    GLO = [12, 8, 4, 0]
    GHI = [15, 12, 8, 4]

    def gran_of(k):                                # granule for pair (k, k+1)
        for g in range(4):
            if k >= GLO[g]:
                return g, k - GLO[g]
        raise AssertionError

    with tile.TileContext(nc) as tc:
        with (
            tc.tile_pool(name="xs", bufs=1) as xpool,
            tc.tile_pool(name="w", bufs=1) as wpool,
            tc.tile_pool(name="psum", bufs=5, space="PSUM") as psum_pool,
            tc.tile_pool(name="gsum", bufs=3, space="PSUM") as gsum_pool,
            tc.tile_pool(name="out", bufs=8) as out_pool,
        ):
            load_eng = [nc.sync, nc.scalar, nc.gpsimd]

            xg = [
                xpool.tile([128, (GHI[g] - GLO[g] + 1) * KSTR], f8,
                           tag=f"xg{g}", name=f"xg{g}")
                for g in range(4)
            ]
            # tci=0 needs k15 + r1 + rc: one per queue, first in line
            nc.sync.dma_start(
                xg[0][:, 3 * KSTR : 4 * KSTR], xall_t.ap()[:, 15 * KSTR :]
            )
            r1 = wpool.tile([128, 2 * M], f8, tag="r1")
            nc.scalar.dma_start(r1[:], r1_t.ap())
            rc = wpool.tile([128, 2 * M], f8, tag="rc")
            nc.gpsimd.dma_start(rc[:], rc_t.ap())
            r2 = wpool.tile([128, 2 * M], f8, tag="r2")
            nc.scalar.dma_start(r2[:], r2_t.ap())
            ut = wpool.tile([128, 2 * RANK], f8, tag="ut")
            nc.gpsimd.dma_start(ut[:], ut_t.ap())
            rb = wpool.tile([128, 2 * M], f8, tag="rb")
            nc.sync.dma_start(rb[:], rb_t.ap())

            # remaining slot loads in consumption order, software-pipelined
            # into the b=0 group loop below
            loads = []
            for g in range(4):
                hi = GHI[g] - GLO[g] if g > 0 else GHI[g] - GLO[g] - 1
                for lo in range(hi, -1, -1):
                    loads.append((g, lo))
            li = 0

            def issue_load(n):
                nonlocal li
                for _ in range(n):
                    if li >= len(loads):
                        return
                    g, lo = loads[li]
                    a = lo * KSTR
                    da = (GLO[g] + lo) * KSTR
                    load_eng[li % 3].dma_start(
                        xg[g][:, a : a + KSTR], xall_t.ap()[:, da : da + KSTR]
                    )
                    li += 1

            issue_load(len(loads))                  # issue all, in order

            xgr = [
                xg[g][:].rearrange(
                    "p (k kind col b) -> p k kind col b",
                    k=GHI[g] - GLO[g] + 1, kind=2, col=128, b=BLOC,
                )
                for g in range(4)
            ]
            r1v = r1[:].rearrange("p (s m) -> p s m", s=2)
            r2v = r2[:].rearrange("p (s m) -> p s m", s=2)
            rcv = rc[:].rearrange("p (s m) -> p s m", s=2)
            utv = ut[:].rearrange("p (s r) -> p s r", s=2)
            rbv = rb[:].rearrange("p (s m) -> p s m", s=2)

            gi = 0
            ot = [None] * BLOC
            pending = None                          # (ps, cmbv, b, tci, gi)

            def evac(ps, pb, ptci, pgi):
                c = ptci % SC
                if c == 0:
                    ot[pb] = out_pool.tile([128, SC * M], f16, name=f"ot{pb}")
                # PSUM->SBUF fp16 downcast, alternate DVE/ACT
                if (pgi % 2) == 0:
                    nc.vector.tensor_copy(ot[pb][:, c * M : (c + 1) * M], ps[:])
                else:
                    nc.scalar.copy(ot[pb][:, c * M : (c + 1) * M], ps[:])
                if c == 1:
                    q = ptci // SC
                    eng = nc.sync if (pgi // 2) % 2 == 0 else nc.scalar
                    eng.dma_start(y_t.ap()[pb, q, :, : 2 * M],
                                  ot[pb][:, : 2 * M])
                elif c == SC - 1:
                    q = ptci // SC
                    eng = nc.sync if (pgi // 2) % 2 == 0 else nc.scalar
                    eng.dma_start(y_t.ap()[pb, q, :, 2 * M :],
                                  ot[pb][:, 2 * M :])

            def finish_pending():
                nonlocal pending
                if pending is None:
                    return
                ps, cmbv, pb, ptci, pgi = pending
                pending = None
                # MM_B: slot0 = x_lo (full compA), slot1 = g|x_hi-half
                # against rb = [Hf0_hi | V,Hf0_lo-half]
                nc.tensor.matmul(
                    ps[:], lhsT=cmbv, rhs=rbv,
                    start=False, stop=True, perf_mode=DR,
                )
                evac(ps, pb, ptci, pgi)

            for b in range(BLOC):
                for tci in range(TCH):
                    k1 = (NREAL - 1) - tci          # lag tiles (0,1)
                    k2 = k1 + 2                     # lag tiles (2,3)
                    g1, l1 = gran_of(k1)
                    ps = psum_pool.tile([128, M], f32)
                    if tci == 0:                    # lag1 slice is zero pad
                        nc.tensor.matmul(
                            ps[:], lhsT=xgr[g1][:, l1, 0, :, b],
                            rhs=r1v[:, 0, :], start=True, stop=False,
                        )
                    else:
                        nc.tensor.matmul(
                            ps[:], lhsT=xgr[g1][:, l1 : l1 + 2, 0, :, b],
                            rhs=r1v, start=True, stop=False, perf_mode=DR,
                        )
                    if tci < 3:
                        # early chunks: far tail mostly zero padding; use
                        # the plain comp pair (and single lag-2 MM at tci=2)
                        if tci == 2:
                            g2, l2 = gran_of(NREAL - 1)
                            nc.tensor.matmul(
                                ps[:], lhsT=xgr[g2][:, l2, 0, :, b],
                                rhs=r2v[:, 0, :], start=False, stop=False,
                            )
                        finish_pending()
                        nc.tensor.matmul(
                            ps[:], lhsT=xgr[g1][:, l1, 0:2, :, b],
                            rhs=rcv, start=False, stop=True, perf_mode=DR,
                        )
                        evac(ps, b, tci, gi)
                    else:
                        # far tail (lags 256..511) via rank-RANK projection:
                        # g = U^T-conv, then V-spread shares MM_B's slot1
                        g2, l2 = gran_of(k2)
                        gps = gsum_pool.tile([RANK, 128], f32)
                        nc.tensor.matmul(
                            gps[:], lhsT=utv,
                            rhs=xgr[g2][:, l2 : l2 + 2, 0, :, b],
                            start=True, stop=True, perf_mode=DR,
                        )
                        nc.vector.tensor_scalar_mul(
                            xgr[g1][:RANK, l1, 1, :, b], gps[:], 2.0 ** -7
                        )
                        finish_pending()
                        pending = (ps, xgr[g1][:, l1, 0:2, :, b], b, tci, gi)
                    gi += 1
            finish_pending()

    nc.compile()
    return nc


def _impulse_f64(lnl_re, lnl_im, W_r, W_i, C, D):
    lnl = lnl_re.astype(np.float64) + 1j * lnl_im.astype(np.float64)
    W = W_r.astype(np.float64) + 1j * W_i.astype(np.float64)
    Winv = np.linalg.inv(W)
    A_re = np.ascontiguousarray(Winv.real.T) @ C.astype(np.float64)
    A_im = np.ascontiguousarray(Winv.imag.T) @ C.astype(np.float64)
    j = np.arange(NLAG * 128, dtype=np.float64)
    P = np.exp(np.outer(j, lnl))
    H = P.real @ A_re - P.imag @ A_im
    H[0] += D[0].astype(np.float64)
    return H                                        # (NLAG*128, M) float64


def host_weights(lnl_re, lnl_im, W_r, W_i, C, D, Do):
    """fp8 moving operands: hi lag pairs, Dekker pairs, rank-RANK tail."""
    H = _impulse_f64(lnl_re, lnl_im, W_r, W_i, C, D)
    H_hi8 = (H * SH).astype(np.float32).astype(E4)
    H_lo8 = ((H[:128] * SH).astype(np.float32) - H_hi8[:128].astype(np.float32)).astype(E4)

    def flip(tile8):                                # lag-flip within a tile
        return np.ascontiguousarray(tile8[::-1, :])

    hf = [flip(H_hi8[128 * lg : 128 * (lg + 1)]) for lg in range(NLAG)]
    hfl0 = flip(H_lo8)
    r1 = np.ascontiguousarray(np.concatenate([hf[0], hf[1]], axis=1))
    r2 = np.ascontiguousarray(np.concatenate([hf[2], hf[3]], axis=1))
    rc = np.ascontiguousarray(np.concatenate([hfl0, hf[0]], axis=1))

    # far tail (lags 256..511) rank-RANK SVD factors, fp8 operands:
    # U_op = fp8(64*U) flipped per 128-lag tile and paired for DoubleRow;
    # rb slot1 = [fp8(32*S*Vt) ; Hf0_lo rows RANK:128] pairs with the
    # device-assembled [g ; x_hi] mixed stationary.
    U, S, Vt = np.linalg.svd(H[256:512], full_matrices=False)
    U_op = (U[:, :RANK] * 64.0).astype(np.float32).astype(E4)   # (256, RANK)
    V_op = (32.0 * S[:RANK, None] * Vt[:RANK]).astype(np.float32).astype(E4)
    uf = [flip(U_op[128 * i : 128 * (i + 1)]) for i in range(2)]
    ut = np.ascontiguousarray(np.concatenate(uf, axis=1))       # (128, 2*RANK)
    rb_s1 = np.concatenate([V_op, hf[0][RANK:]], axis=0)
    rb = np.ascontiguousarray(np.concatenate([hfl0, rb_s1], axis=1))
    return {"r1": r1, "r2": r2, "rc": rc, "ut": ut, "rb": rb}


def make_in_maps(x, weights):
    """Per-core input dict: packed diagonalized hi/lo x + H tiles."""
    x64 = x[:, :, 0].astype(np.float32)             # (B, T)
    xh8 = (x64 * SX).astype(E4)
    xl8 = (x64 * SX - xh8.astype(np.float32)).astype(E4)

    # gather index A[k, p, col] = 128*(18-k) + p + col  into xpad rows
    k = np.arange(NREAL)
    A = (128 * (ND - 1 - k))[:, None, None] + np.arange(128)[None, :, None] \
        + np.arange(128)[None, None, :]             # (16, 128, 128)

    in_maps = []
    for c in range(NCORES):
        sl = slice(c * BLOC, (c + 1) * BLOC)
        xpad = np.zeros((2, XPLEN, BLOC), E4)       # [kind][row][b]
        xpad[0, RPAD:, :] = xh8[sl].T
        xpad[1, RPAD:, :] = xl8[sl].T
        g = xpad[:, A, :]                           # (2, 16, 128, 128, BLOC)
        xa = np.ascontiguousarray(np.transpose(g, (2, 1, 0, 3, 4)))
        im = dict(weights)
        im["xall"] = xa.reshape(128, NREAL * 2 * 128 * BLOC)
        in_maps.append(im)
    return in_maps


_prog_cache = {}


def kernel(x, lnl_re, lnl_im, W_r, W_i, C, D, Do):
    from concourse.bass_utils import run_bass_kernel_spmd

    x = np.asarray(x)
    lnl_re, lnl_im = np.asarray(lnl_re), np.asarray(lnl_im)
    W_r, W_i = np.asarray(W_r), np.asarray(W_i)
    C, D, Do = np.asarray(C), np.asarray(D), np.asarray(Do)

    key = (NLAG, MODE)
    if key not in _prog_cache:
        _prog_cache[key] = build_program()
    nc = _prog_cache[key]

    weights = host_weights(lnl_re, lnl_im, W_r, W_i, C, D, Do)
    in_maps = make_in_maps(np.asarray(x, np.float32), weights)
    res = run_bass_kernel_spmd(nc, in_maps, core_ids=list(range(NCORES)))
    # device layout [bloc, q, p, c*M]: t = 512*q + 128*c + p
    y = np.concatenate([res.results[i]["y"] for i in range(NCORES)], axis=0)
    y = y.reshape(B, TCH // 4, 128, 4, M).transpose(0, 1, 3, 2, 4).reshape(B, T, M)
    y = y.astype(np.float32) * np.float32(1.0 / (SX * SH)) + Do.astype(np.float32)
    return np.ascontiguousarray(y.astype(np.float32))


# revision 32
# speedup vs baseline: 2.2588x; 1.0539x over previous
"""Trainium2 Bass kernel for nn_LuenbergerLDS (B=32, T=2048, N=512, M=512).

Math: the reference is a diagonal complex linear recurrence
    s_t = lam * s_{t-1} + x_t   (per batch, per n; x scalar per t broadcast)
followed by  y = Re(Winv @ s) @ C + x @ D + Do.

Since d == 1 the whole module is a causal LTI SIMO filter:
    y[t, b, m] = sum_{j>=0} H[j, m] * x[t - j, b] + Do[m]
with impulse response (computed on host in float64)
    H[j, m] = sum_n Re(lam_n^j) * A_re[n, m] - Im(lam_n^j) * A_im[n, m]
    A_re = Re(Winv)^T @ C,  A_im = Im(Winv)^T @ C,  H[0] += D.
A window of NLAG*128 = 512 lags truncates at 8.5e-4 of max|y| (measured
exactly on the reference data; gate is 2e-2).

Device work (per core, data-parallel over batch: 4 batches/core): pure
fp8(e4m3) matmuls in DoubleRow perf mode (2 contraction slots per
partition, 0.5 cycles/row -> 2x f32r throughput). For output chunk
t0..t0+127, the stationary operand packs TWO lag tiles' Toeplitz
diagonal slices of a zero-padded, pre-diagonalized x buffer (built on
host, so DMA loads are contiguous); the moving operand packs the two
matching row-flipped H tiles (128x(2*512)). 3 DoubleRow matmuls per
chunk: lags(0,1)_hi, lags(2,3)_hi, and a Dekker compensation pair
(x_lo*H_hi0 + x_hi*H_lo0) that fixes the head tile's fp8 quantization.

Scaling: x ops are x*32, H ops are H*16, so PSUM holds y*512 (absmax
39.3k < fp16 max). Device just downcasts PSUM to fp16 and stores; the
host divides by 512, adds Do, and upcasts to f32. Measured end-to-end
error of this exact pipeline on the reference data: 3.9e-3.
"""

import sys

sys.path.insert(0, "/opt/trn_rl_repo")

import numpy as np
import ml_dtypes

E4 = ml_dtypes.float8_e4m3

# problem dims (hardcoded per harness contract)
B, T, N, M = 32, 2048, 512, 512
NCORES = 8
BLOC = B // NCORES          # batches per core
NLAG = 4                    # lag window = NLAG*128 = 512
MODE = "fp8dr"
SX, SH = 32.0, 16.0         # operand scales; PSUM = y * SX*SH

RPAD = 128 * NLAG - 1       # 511 zero rows ahead of x in xpad
XPLEN = RPAD + T            # 2559
ND = (T + 128 * NLAG - 128) // 128   # 19 diagonal slices (d=0..18)
NZERO = NLAG - 1            # slices d<3 are all zero padding
NREAL = ND - NZERO          # 16 slices carried in DRAM (k=0..15)
TCH = T // 128              # 16 output chunks per batch
RANK = 32                   # SVD rank of the far-tail (lags 256..511)
NCB = 128 - RANK            # compB coverage: H0_lo lags 0..NCB-1


def build_program():
    """Build + compile the (SPMD, per-core) Bass program."""
    import concourse.tile as tile
    from concourse import bacc, mybir

    f32 = mybir.dt.float32
    f16 = mybir.dt.float16
    f8 = mybir.dt.float8e4
    DR = mybir.MatmulPerfMode.DoubleRow

    # free-dim element counts of the packed x tensor: [k, kind, col, b]
    # kind: 0 = x_hi, 1 = x_lo. For slots k<=12 the partitions 0:RANK of
    # kind1 are dead (compA only covers lags 0..95) and get overwritten
    # in-place by the per-group g cast, so the MM_B stationary pair
    # (x_hi, g|x_lo) is a single AP with no SBUF copies.
    KSTR = 2 * 128 * BLOC   # 1024 elements per k slot
    SC = 4                  # output chunks batched per store (4KB DMA lines)
    nc = bacc.Bacc("TRN2", target_bir_lowering=False, debug=False)
    xall_t = nc.dram_tensor("xall", [128, NREAL * KSTR], f8, kind="ExternalInput")
    r1_t = nc.dram_tensor("r1", [128, 2 * M], f8, kind="ExternalInput")
    r2_t = nc.dram_tensor("r2", [128, 2 * M], f8, kind="ExternalInput")
    rc_t = nc.dram_tensor("rc", [128, 2 * M], f8, kind="ExternalInput")
    ut_t = nc.dram_tensor("ut", [128, 2 * RANK], f8, kind="ExternalInput")
    rb_t = nc.dram_tensor("rb", [128, 2 * M], f8, kind="ExternalInput")
    # y stored chunk-batched: [b, q, p, c*M]; host untransposes
    y_t = nc.dram_tensor(
        "y", [BLOC, TCH // SC, 128, SC * M], f16, kind="ExternalOutput"
    )

    # SBUF granules over k so dependency tracking is fine-grained and a
    # matmul's (k, k+1) pair never crosses a granule: granule g covers
    # k in [GLO[g], GHI[g]] inclusive, boundary slots duplicated.
    # Slices k>15 (d<3) are all-zero: their matmuls are skipped instead.
    GLO = [12, 8, 4, 0]
    GHI = [15, 12, 8, 4]

    def gran_of(k):                                # granule for pair (k, k+1)
        for g in range(4):
            if k >= GLO[g]:
                return g, k - GLO[g]
        raise AssertionError

    with tile.TileContext(nc) as tc:
        with (
            tc.tile_pool(name="xs", bufs=1) as xpool,
            tc.tile_pool(name="w", bufs=1) as wpool,
            tc.tile_pool(name="psum", bufs=5, space="PSUM") as psum_pool,
            tc.tile_pool(name="gsum", bufs=3, space="PSUM") as gsum_pool,
            tc.tile_pool(name="out", bufs=8) as out_pool,
        ):
            load_eng = [nc.sync, nc.scalar, nc.gpsimd]

            xg = [
                xpool.tile([128, (GHI[g] - GLO[g] + 1) * KSTR], f8,
                           tag=f"xg{g}", name=f"xg{g}")
                for g in range(4)
            ]
            # tci=0 needs k15 + r1 + rc: one per queue, first in line
            nc.sync.dma_start(
                xg[0][:, 3 * KSTR : 4 * KSTR], xall_t.ap()[:, 15 * KSTR :]
            )
            r1 = wpool.tile([128, 2 * M], f8, tag="r1")
            nc.scalar.dma_start(r1[:], r1_t.ap())
            rc = wpool.tile([128, 2 * M], f8, tag="rc")
            nc.gpsimd.dma_start(rc[:], rc_t.ap())
            r2 = wpool.tile([128, 2 * M], f8, tag="r2")
            nc.scalar.dma_start(r2[:], r2_t.ap())
            ut = wpool.tile([128, 2 * RANK], f8, tag="ut")
            nc.gpsimd.dma_start(ut[:], ut_t.ap())
            rb = wpool.tile([128, 2 * M], f8, tag="rb")
            nc.sync.dma_start(rb[:], rb_t.ap())

            # remaining slot loads in consumption order, software-pipelined
            # into the b=0 group loop below
            loads = []
            for g in range(4):
                hi = GHI[g] - GLO[g] if g > 0 else GHI[g] - GLO[g] - 1
                for lo in range(hi, -1, -1):
                    loads.append((g, lo))
            li = 0

            def issue_load(n):
                nonlocal li
                for _ in range(n):
                    if li >= len(loads):
                        return
                    g, lo = loads[li]
                    a = lo * KSTR
                    da = (GLO[g] + lo) * KSTR
                    load_eng[li % 3].dma_start(
                        xg[g][:, a : a + KSTR], xall_t.ap()[:, da : da + KSTR]
                    )
                    li += 1

            issue_load(len(loads))                  # issue all, in order

            xgr = [
                xg[g][:].rearrange(
                    "p (k kind col b) -> p k kind col b",
                    k=GHI[g] - GLO[g] + 1, kind=2, col=128, b=BLOC,
                )
                for g in range(4)
            ]
            r1v = r1[:].rearrange("p (s m) -> p s m", s=2)
            r2v = r2[:].rearrange("p (s m) -> p s m", s=2)
            rcv = rc[:].rearrange("p (s m) -> p s m", s=2)
            utv = ut[:].rearrange("p (s r) -> p s r", s=2)
            rbv = rb[:].rearrange("p (s m) -> p s m", s=2)

            gi = 0
            ot = [None] * BLOC
            pending = None                          # (ps, cmbv, b, tci, gi)

            def evac(ps, pb, ptci, pgi):
                c = ptci % SC
                if c == 0:
                    ot[pb] = out_pool.tile([128, SC * M], f16, name=f"ot{pb}")
                # PSUM->SBUF fp16 downcast, alternate DVE/ACT
                if (pgi % 2) == 0:
                    nc.vector.tensor_copy(ot[pb][:, c * M : (c + 1) * M], ps[:])
                else:
                    nc.scalar.copy(ot[pb][:, c * M : (c + 1) * M], ps[:])
                if c == 1:
                    q = ptci // SC
                    eng = nc.sync if (pgi // 2) % 2 == 0 else nc.scalar
                    eng.dma_start(y_t.ap()[pb, q, :, : 2 * M],
                                  ot[pb][:, : 2 * M])
                elif c == SC - 1:
                    q = ptci // SC
                    eng = nc.sync if (pgi // 2) % 2 == 0 else nc.scalar
                    eng.dma_start(y_t.ap()[pb, q, :, 2 * M :],
                                  ot[pb][:, 2 * M :])

            def finish_pending():
                nonlocal pending
                if pending is None:
                    return
                ps, cmbv, pb, ptci, pgi = pending
                pending = None
                # MM_B: slot0 = x_lo (full compA), slot1 = g|x_hi-half
                # against rb = [Hf0_hi | V,Hf0_lo-half]
                nc.tensor.matmul(
                    ps[:], lhsT=cmbv, rhs=rbv,
                    start=False, stop=True, perf_mode=DR,
                )
                evac(ps, pb, ptci, pgi)

            for tci in range(TCH):
                for b in range(BLOC):
                    k1 = (NREAL - 1) - tci          # lag tiles (0,1)
                    k2 = k1 + 2                     # lag tiles (2,3)
                    g1, l1 = gran_of(k1)
                    ps = psum_pool.tile([128, M], f32)
                    if tci == 0:                    # lag1 slice is zero pad
                        nc.tensor.matmul(
                            ps[:], lhsT=xgr[g1][:, l1, 0, :, b],
                            rhs=r1v[:, 0, :], start=True, stop=False,
                        )
                    else:
                        nc.tensor.matmul(
                            ps[:], lhsT=xgr[g1][:, l1 : l1 + 2, 0, :, b],
                            rhs=r1v, start=True, stop=False, perf_mode=DR,
                        )
                    if tci < 3:
                        # early chunks: far tail mostly zero padding; use
                        # the plain comp pair (and single lag-2 MM at tci=2)
                        if tci == 2:
                            g2, l2 = gran_of(NREAL - 1)
                            nc.tensor.matmul(
                                ps[:], lhsT=xgr[g2][:, l2, 0, :, b],
                                rhs=r2v[:, 0, :], start=False, stop=False,
                            )
                        finish_pending()
                        nc.tensor.matmul(
                            ps[:], lhsT=xgr[g1][:, l1, 0:2, :, b],
                            rhs=rcv, start=False, stop=True, perf_mode=DR,
                        )
                        evac(ps, b, tci, gi)
                    else:
                        # far tail (lags 256..511) via rank-RANK projection:
                        # g = U^T-conv, then V-spread shares MM_B's slot1
                        g2, l2 = gran_of(k2)
                        gps = gsum_pool.tile([RANK, 128], f32)
                        nc.tensor.matmul(
                            gps[:], lhsT=utv,
                            rhs=xgr[g2][:, l2 : l2 + 2, 0, :, b],
                            start=True, stop=True, perf_mode=DR,
                        )
                        nc.vector.tensor_scalar_mul(
                            xgr[g1][:RANK, l1, 1, :, b], gps[:], 2.0 ** -7
                        )
                        finish_pending()
                        pending = (ps, xgr[g1][:, l1, 0:2, :, b], b, tci, gi)
                    gi += 1
            finish_pending()

    nc.compile()
    return nc


def _impulse_f64(lnl_re, lnl_im, W_r, W_i, C, D):
    lnl = lnl_re.astype(np.float64) + 1j * lnl_im.astype(np.float64)
    W = W_r.astype(np.float64) + 1j * W_i.astype(np.float64)
    Winv = np.linalg.inv(W)
    A_re = np.ascontiguousarray(Winv.real.T) @ C.astype(np.float64)
    A_im = np.ascontiguousarray(Winv.imag.T) @ C.astype(np.float64)
    j = np.arange(NLAG * 128, dtype=np.float64)
    P = np.exp(np.outer(j, lnl))
    H = P.real @ A_re - P.imag @ A_im
    H[0] += D[0].astype(np.float64)
    return H                                        # (NLAG*128, M) float64


def host_weights(lnl_re, lnl_im, W_r, W_i, C, D, Do):
    """fp8 moving operands: hi lag pairs, Dekker pairs, rank-RANK tail."""
    H = _impulse_f64(lnl_re, lnl_im, W_r, W_i, C, D)
    H_hi8 = (H * SH).astype(np.float32).astype(E4)
    H_lo8 = ((H[:128] * SH).astype(np.float32) - H_hi8[:128].astype(np.float32)).astype(E4)

    def flip(tile8):                                # lag-flip within a tile
        return np.ascontiguousarray(tile8[::-1, :])

    hf = [flip(H_hi8[128 * lg : 128 * (lg + 1)]) for lg in range(NLAG)]
    hfl0 = flip(H_lo8)
    r1 = np.ascontiguousarray(np.concatenate([hf[0], hf[1]], axis=1))
    r2 = np.ascontiguousarray(np.concatenate([hf[2], hf[3]], axis=1))
    rc = np.ascontiguousarray(np.concatenate([hfl0, hf[0]], axis=1))

    # far tail (lags 256..511) rank-RANK SVD factors, fp8 operands:
    # U_op = fp8(64*U) flipped per 128-lag tile and paired for DoubleRow;
    # rb slot1 = [fp8(32*S*Vt) ; Hf0_lo rows RANK:128] pairs with the
    # device-assembled [g ; x_hi] mixed stationary.
    U, S, Vt = np.linalg.svd(H[256:512], full_matrices=False)
    U_op = (U[:, :RANK] * 64.0).astype(np.float32).astype(E4)   # (256, RANK)
    V_op = (32.0 * S[:RANK, None] * Vt[:RANK]).astype(np.float32).astype(E4)
    uf = [flip(U_op[128 * i : 128 * (i + 1)]) for i in range(2)]
    ut = np.ascontiguousarray(np.concatenate(uf, axis=1))       # (128, 2*RANK)
    rb_s1 = np.concatenate([V_op, hf[0][RANK:]], axis=0)
    rb = np.ascontiguousarray(np.concatenate([hfl0, rb_s1], axis=1))
    return {"r1": r1, "r2": r2, "rc": rc, "ut": ut, "rb": rb}


def make_in_maps(x, weights):
    """Per-core input dict: packed diagonalized hi/lo x + H tiles."""
    x64 = x[:, :, 0].astype(np.float32)             # (B, T)
    xh8 = (x64 * SX).astype(E4)
    xl8 = (x64 * SX - xh8.astype(np.float32)).astype(E4)

    # gather index A[k, p, col] = 128*(18-k) + p + col  into xpad rows
    k = np.arange(NREAL)
    A = (128 * (ND - 1 - k))[:, None, None] + np.arange(128)[None, :, None] \
        + np.arange(128)[None, None, :]             # (16, 128, 128)

    in_maps = []
    for c in range(NCORES):
        sl = slice(c * BLOC, (c + 1) * BLOC)
        xpad = np.zeros((2, XPLEN, BLOC), E4)       # [kind][row][b]
        xpad[0, RPAD:, :] = xh8[sl].T
        xpad[1, RPAD:, :] = xl8[sl].T
        g = xpad[:, A, :]                           # (2, 16, 128, 128, BLOC)
        xa = np.ascontiguousarray(np.transpose(g, (2, 1, 0, 3, 4)))
        im = dict(weights)
        im["xall"] = xa.reshape(128, NREAL * 2 * 128 * BLOC)
        in_maps.append(im)
    return in_maps


_prog_cache = {}


def kernel(x, lnl_re, lnl_im, W_r, W_i, C, D, Do):
    from concourse.bass_utils import run_bass_kernel_spmd

    x = np.asarray(x)
    lnl_re, lnl_im = np.asarray(lnl_re), np.asarray(lnl_im)
    W_r, W_i = np.asarray(W_r), np.asarray(W_i)
    C, D, Do = np.asarray(C), np.asarray(D), np.asarray(Do)

    key = (NLAG, MODE)
    if key not in _prog_cache:
        _prog_cache[key] = build_program()
    nc = _prog_cache[key]

    weights = host_weights(lnl_re, lnl_im, W_r, W_i, C, D, Do)
    in_maps = make_in_maps(np.asarray(x, np.float32), weights)
    res = run_bass_kernel_spmd(nc, in_maps, core_ids=list(range(NCORES)))
    # device layout [bloc, q, p, c*M]: t = 512*q + 128*c + p
    y = np.concatenate([res.results[i]["y"] for i in range(NCORES)], axis=0)
    y = y.reshape(B, TCH // 4, 128, 4, M).transpose(0, 1, 3, 2, 4).reshape(B, T, M)
    y = y.astype(np.float32) * np.float32(1.0 / (SX * SH)) + Do.astype(np.float32)
    return np.ascontiguousarray(y.astype(np.float32))
